# revision 2
# baseline (speedup 1.0000x reference)
"""Correlation (cost volume) kernel for Trainium2, 8-core data parallel. v2.

Math (matches the reference):
  out[b, di*9+dj, i, j] = (proj(x1)[b,:,i,j] . proj(x2)[b,:,i+di-4,j+dj-4]) / sqrt(128)
Rewritten with M = (W^T W)/sqrt(128) (symmetric):
  out[b, d, i, j] = x1[b,:,i,j] . z2p[b,:,i+di,j+dj],  z2p = zero-pad(M @ x2)
(bias handled on host as a rank-1 correction; proj_b is zero in practice).

Device strategy (per core, 4 batches):
  - z2p = M @ x2 as chunked [128x128]@[128,384] matmuls into a padded bf16
    image [128, 104, 104] (pad frame memset on GPSIMD). x2 rows 48..95
    arrive as fp8-e3m4 (1.4% elem rms), rows 0..47 as bf16 - halves input
    DMA bytes for that image at ~1% output error.
  - correlation: x1 in 16x8-pixel blocks (72/batch). Per block one matmul:
    stationary = x1 block [128c, 128px] bf16, moving = z2p region
    [128c, 24, 16] -> PSUM [128, 384] fp32.
  - PSUM tiles are [128, 1024] fp32 (2 banks) holding 2 blocks/chunks at
    column 0 and 512, so one DVE/ACT copy drains 2 matmuls (halves the
    per-instruction SBUF/PSUM access-latency tax). Copies convert to
    fp8-e3m4 (band values are O(1) after the 1/sqrt(128) fold in M).
    DVE/ACT alternation is balanced by modeled busy time incl overheads.
  - band DMA per block-row: [128, 12*384] e3m4, contiguous both sides.
  - host: decode e3m4, gather out[d,i,j] = band[gr,u*8+v,gc,(u+di)*16+(v+dj)].
"""
import math

import numpy as np
import ml_dtypes

import concourse.bass as bass
import concourse.bacc as bacc
import concourse.tile as tile
import concourse.mybir as mybir
from concourse.bass_utils import run_bass_kernel_spmd

B, C, H, W = 32, 128, 96, 96
NCORES = 8
BLOC = B // NCORES          # 4 batches per core
PATCH = 9
R = PATCH // 2              # 4
PH, PW = H + 2 * R, W + 2 * R  # 104 x 104 padded
NPIX = H * W                # 9216
PCHUNK = 384                # projection chunk (4 image rows)
NCHUNK = NPIX // PCHUNK     # 24
NC2 = NCHUNK // 2           # 12 chunks per x2 precision half
BH, BW = 16, 8              # x1 stationary block (128 px)
GR, GC = H // BH, W // BW   # 6 x 12 block grid
RH, RW = BH + 2 * R, BW + 2 * R  # 24 x 16 moving region
REG = RH * RW               # 384 psum cols per block
PAIR = 1024                 # 2-bank psum pair tile (fp32 cols)
BAND_DT = mybir.dt.float8e3

_cache: dict = {}


def _build_program():
    nc = bacc.Bacc(target_bir_lowering=False)
    bf = mybir.dt.bfloat16
    f32 = mybir.dt.float32

    x1d = nc.declare_dram_parameter("x1", [BLOC, C, NPIX], bf, isOutput=False)
    x2ad = nc.declare_dram_parameter("x2a", [BLOC, C, NC2 * PCHUNK], bf,
                                     isOutput=False)
    x2bd = nc.declare_dram_parameter("x2b", [BLOC, C, NC2 * PCHUNK],
                                     mybir.dt.float8e3, isOutput=False)
    md = nc.declare_dram_parameter("m", [C, C], bf, isOutput=False)
    bandd = nc.declare_dram_parameter(
        "band", [BLOC, GR, BH * BW, GC * REG], BAND_DT, isOutput=True
    )

    with tile.TileContext(nc) as tc:
        with (
            tc.tile_pool(name="consts", bufs=1) as consts,
            tc.tile_pool(name="imgs", bufs=2) as imgs,
            tc.tile_pool(name="feats", bufs=2) as feats,
            tc.tile_pool(name="bands", bufs=6) as bands,
            tc.tile_pool(name="pps", bufs=2, space="PSUM") as pps,
            tc.tile_pool(name="bps", bufs=3, space="PSUM") as bps,
        ):
            mt = consts.tile([C, C], bf, tag="m")
            nc.sync.dma_start(out=mt[:, :], in_=md[:, :])



            # balance PSUM->SBUF copies across DVE/ACT by modeled busy ns
            # (DVE: 1.0417 ns/col + 125 ns/inst; ACT: 0.8333 + 185)
            state = {"dve": 0.0, "act": 0.0}

            def copy(dst, src, ncols):
                if state["dve"] + ncols * 1.0417 + 125 <= \
                        state["act"] + ncols * 0.8333 + 185:
                    state["dve"] += ncols * 1.0417 + 125
                    nc.vector.tensor_copy(dst, src)
                else:
                    state["act"] += ncols * 0.8333 + 185
                    nc.scalar.copy(dst, src)

            for b in range(BLOC):
                # inputs go through the Pool (SWDGE) DMA queue so they never
                # queue behind band-output DMAs on SP's in-order sequencer;
                # x2 first (projection consumes it before x1 is needed)
                x2a = imgs.tile([C, NC2 * PCHUNK], bf, tag="x2a")
                nc.gpsimd.dma_start(out=x2a[:, :], in_=x2ad[b, :, :])
                x2b = imgs.tile([C, NC2 * PCHUNK], mybir.dt.float8e3,
                                tag="x2b")
                nc.gpsimd.dma_start(out=x2b[:, :], in_=x2bd[b, :, :])
                # x1 arrives host-blocked: [C, 72 blocks, 128 px]
                x1t = imgs.tile([C, NPIX], bf, tag="x1")
                nc.gpsimd.dma_start(out=x1t[:, :], in_=x1d[b, :, :])

                z2 = feats.tile([C, PH, PW], bf, tag="z2")
                if b < 2:
                    # zero the pad frame once per physical buffer (bufs=2);
                    # later batches only rewrite the interior, the frame
                    # stays zero
                    nc.gpsimd.memset(z2[:, 0:R, :], 0.0)
                    nc.gpsimd.memset(z2[:, R + H:PH, :], 0.0)
                    nc.gpsimd.memset(z2[:, R:R + H, 0:R], 0.0)
                    nc.gpsimd.memset(z2[:, R:R + H, R + W:PW], 0.0)

                # projection: one chunk per single-bank psum tile
                for k in range(NCHUNK):
                    p = pps.tile([C, PCHUNK], f32, tag="pp")
                    rhs = (x2a[:, bass.ts(k, PCHUNK)] if k < NC2
                           else x2b[:, bass.ts(k - NC2, PCHUNK)])
                    nc.tensor.matmul(p[:, :], mt[:, :], rhs,
                                     start=True, stop=True)
                    pv = p[:, :].rearrange("c (r w) -> c r w", w=W)
                    copy(z2[:, R + 4 * k:R + 4 * k + 4, R:R + W], pv, PCHUNK)

                for gr in range(GR):
                    bt = bands.tile([BH * BW, GC * REG], BAND_DT, tag="bt")
                    btv = bt[:, :].rearrange("p (g c) -> p g c", c=REG)
                    for gt in range(GC // 2):
                        pb = bps.tile([BH * BW, PAIR], f32, tag="pb")
                        for t in range(2):
                            gc = 2 * gt + t
                            nc.tensor.matmul(
                                pb[:, 512 * t:512 * t + REG],
                                x1t[:, bass.ts(gr * GC + gc, BH * BW)],
                                z2[:, BH * gr:BH * gr + RH,
                                   BW * gc:BW * gc + RW],
                                start=True, stop=True,
                            )
                        pv = pb[:, :].rearrange("p (t c) -> p t c", t=2)[
                            :, :, 0:REG]
                        copy(btv[:, 2 * gt:2 * gt + 2, :], pv, 2 * REG)
                    nc.sync.dma_start(out=bandd[b, gr, :, :], in_=bt[:, :])

    nc.compile()
    return nc


def kernel(input1, input2, proj_w, proj_b):
    if "nc" not in _cache:
        _cache["nc"] = _build_program()
    nc = _cache["nc"]

    w64 = np.asarray(proj_w, dtype=np.float64)
    m = (w64.T @ w64) / math.sqrt(C)          # symmetric [C, C]
    mt = np.ascontiguousarray(m).astype(ml_dtypes.bfloat16)

    in_maps = []
    for k in range(NCORES):
        sl = slice(BLOC * k, BLOC * (k + 1))
        # block x1: [BLOC, C, H, W] -> [BLOC, C, GR, BH, GC, BW] -> blocked
        x1b = (np.asarray(input1[sl])
               .reshape(BLOC, C, GR, BH, GC, BW)
               .transpose(0, 1, 2, 4, 3, 5)
               .reshape(BLOC, C, NPIX))
        x2 = np.asarray(input2[sl]).reshape(BLOC, C, NPIX)
        in_maps.append({
            "x1": np.ascontiguousarray(x1b).astype(ml_dtypes.bfloat16),
            "x2a": np.ascontiguousarray(x2[:, :, :NC2 * PCHUNK])
                     .astype(ml_dtypes.bfloat16),
            "x2b": np.ascontiguousarray(x2[:, :, NC2 * PCHUNK:])
                     .astype(ml_dtypes.float8_e3m4),
            "m": mt,
        })

    res = run_bass_kernel_spmd(nc, in_maps, list(range(NCORES)))

    # host gather: out[(di,dj), 16gr+u, 8gc+v] = band[gr, u*8+v, gc, (u+di)*16+(v+dj)]
    di = np.arange(PATCH)
    u = np.arange(BH)
    v = np.arange(BW)
    qidx = ((u[:, None, None, None] + di[None, None, :, None]) * RW
            + (v[None, :, None, None] + di[None, None, None, :]))  # [16,8,9,9]
    qflat = qidx.reshape(1, 1, BH, BW, 1, PATCH * PATCH)
    full = []
    for k in range(NCORES):
        band = np.asarray(res.results[k]["band"], dtype=np.float32)
        band = band.reshape(BLOC, GR, BH, BW, GC, REG)
        sel = np.take_along_axis(band, qflat, axis=5)
        sel = sel.reshape(BLOC, GR, BH, BW, GC, PATCH, PATCH)
        o = sel.transpose(0, 5, 6, 1, 2, 4, 3).reshape(
            BLOC, PATCH * PATCH, H, W)
        full.append(o)
    out = np.concatenate(full, axis=0)
    return _finish(out, proj_w, proj_b, input1, input2)


def _finish(out, proj_w, proj_b, input1, input2):
    if np.any(np.asarray(proj_b) != 0):
        # rank-1 bias corrections: (Wx1+b).(Wx2p+b) = x1^T W^T W x2p
        #   + u.x1 + u.x2p + b.b, with u = W^T b (all / sqrt(C))
        w64 = np.asarray(proj_w, dtype=np.float64)
        b64 = np.asarray(proj_b, dtype=np.float64)
        u_vec = (w64.T @ b64) / math.sqrt(C)
        t1 = np.einsum("bchw,c->bhw", np.asarray(input1, np.float64), u_vec)
        t2 = np.einsum("bchw,c->bhw", np.asarray(input2, np.float64), u_vec)
        t2p = np.pad(t2, ((0, 0), (R, R), (R, R)))
        bb = float(b64 @ b64) / math.sqrt(C)
        for di in range(PATCH):
            for dj in range(PATCH):
                shifted = t2p[:, di:di + H, dj:dj + W]
                out[:, di * PATCH + dj] += (t1 + shifted + bb).astype(
                    np.float32)
    return out
